# revision 35
# baseline (speedup 1.0000x reference)
"""Trainium2 Bass kernel for nn_MoELayer (top-2 MoE, E=8 experts).

Strategy (expert-parallel across 8 NeuronCores):
  - Host computes the (tiny) gate matmul + top-2 + softmax and dispatches
    each token to its two experts' cores. One expert per core.
  - Token capacity per core is fixed at C=1024 (2 PSUM-friendly chunks of
    512). The few overflow tokens of experts routed >1024 tokens (~1% of
    FLOPs) are computed on the host in fp32 and folded into the
    scatter-add, keeping every core's device work identical and minimal.
  - Each core runs a RAW-BASS (hand-synchronized) kernel computing, for
    its expert e and its <=1024 routed tokens:
        out = silu(tok @ W1[e]) @ W2[e]
    with bf16 matmul inputs and fp32 PSUM accumulation.

Why raw bass instead of Tile: Tile attaches a semaphore increment to
EVERY matmul (each accumulation-group member has the PSUM-reading
activation as a descendant).  A PE-engine semaphore update costs ~31ns
on the engine timeline (SEM_PROP_BASE 17 + PE send overhead 14), which
measured as a ~33ns/MM tax (~48us total) in the Tile baseline.  Here
semaphores are placed by hand: one inc per 8-matmul accumulation group.

Layouts (no on-device transposes):
  stage 1:  actT[f, c] = silu( sum_d W1[d, f] * tokT[d, c] )
  stage 2:  outT[d, c] = sum_f W2[f, d] * actT[f, c]

DMA plan: W1 streams on the scalar-engine HWDGE ring in 4 fj-ordered
slabs (so the first matmul only waits for tok chunk 0 + slab 0), while
tokens/W2 stream on the sync-engine ring.  Every input DMA is a single
contiguous-per-partition block (own DRAM tensor per slab) for >=4KB
descriptors (full DMA line rate).  ~55 throwaway warm-up matmuls run
during the initial DMA wait to lift the PE HAM clock-gate to 2.4GHz
before real work starts.
"""

import math
import os
import sys

sys.path.insert(0, "/opt/trn_rl_repo")

import ml_dtypes
import numpy as np

B, T, D, F, E = 2, 2048, 1024, 4096, 8
N = B * T
P = 128
KD = D // P  # 8
KF = F // P  # 32

C = 1024  # device token capacity per expert/core
CH = 512  # chunk width (PSUM bank = 512 fp32)
NCHUNK = C // CH  # 2

# W1 streamed in 16 fj-ordered 256-col slabs (0.5MB each), alternating
# between the two HWDGE rings (scalar gets even slabs, sync odd) — one
# ring sustains only ~170GB/s, which cannot keep ahead of the PE's
# 154GB/s W1 burn once the startup path is fast; two rings give 2x margin
# and the first matmul only waits for tok0 + one 0.5MB slab.
W1_NSLAB = 16
W1_COLS = F // W1_NSLAB  # 256
W2_SLAB = 256  # D-cols per W2 slab (4 slabs)
N_WARM = int(os.environ.get("N_WARM", "100"))

bf16 = ml_dtypes.bfloat16

_nc_cache: dict = {}
LAST_RESULTS = None  # BassKernelResults from the most recent run (for test.py)
TRACE = False


def _w1_slab_of(fj: int) -> int:
    return (fj * P) // W1_COLS


def _build():
    import concourse.mybir as mybir
    from concourse import bacc

    dt = mybir.dt
    nc = bacc.Bacc(None, target_bir_lowering=False)

    # ---- DRAM tensors (each a contiguous block => big DMA descriptors) ----
    toks = [
        nc.dram_tensor(f"tok{c}", [P, KD, CH], dt.bfloat16, kind="ExternalInput")
        for c in range(NCHUNK)
    ]
    w1s = [
        nc.dram_tensor(f"w1s{i}", [P, KD, W1_COLS], dt.bfloat16, kind="ExternalInput")
        for i in range(W1_NSLAB)
    ]
    w2s = [
        nc.dram_tensor(f"w2s{i}", [P, KF, W2_SLAB], dt.bfloat16, kind="ExternalInput")
        for i in range(D // W2_SLAB)
    ]
    out = nc.dram_tensor("out", [D, C], dt.float32, kind="ExternalOutput")

    from contextlib import ExitStack

    stack = ExitStack()
    sb = lambda name, shape, dty: stack.enter_context(nc.sbuf_tensor(name, shape, dty))
    ps = lambda name, shape: stack.enter_context(
        nc.psum_tensor(name, shape, dt.float32)
    )
    sem = lambda name: stack.enter_context(nc.semaphore(name))

    tok_sb = [sb(f"tok_sb{c}", [P, KD, CH], dt.bfloat16) for c in range(NCHUNK)]
    w1_sb = [
        sb(f"w1_sb{i}", [P, KD, W1_COLS], dt.bfloat16) for i in range(W1_NSLAB)
    ]
    w2_sb = [sb(f"w2_sb{i}", [P, KF, W2_SLAB], dt.bfloat16) for i in range(len(w2s))]
    act_sb = sb("act_sb", [P, KF, CH], dt.bfloat16)
    ob_sb = sb("ob_sb", [P, 4, CH], dt.float32)  # 4 rotating output buffers
    warm_sb = sb("warm_sb", [P, P], dt.bfloat16)

    ps1 = [ps(f"ps1_{b}", [P, CH]) for b in range(4)]  # stage-1 banks
    ps2 = [ps(f"ps2_{b}", [P, CH]) for b in range(2)]  # stage-2 banks
    warm_ps = ps("warm_ps", [P, CH])

    # One semaphore per input DMA: a DMA's completion is 16 per-engine
    # increments, and increments of DIFFERENT in-flight DMAs sharing one
    # semaphore interleave under engine skew — a cumulative threshold can
    # fire while an earlier DMA is still partially in flight.  Dedicated
    # sems (wait >= 16) are skew-proof.
    tk_sems = [sem(f"tk_sem{c}") for c in range(NCHUNK)]
    w1_sems = [sem(f"w1_sem{i}") for i in range(W1_NSLAB)]
    w2_sems = [sem(f"w2_sem{i}") for i in range(len(w2s))]
    pe_sem = sem("pe_sem")  # PE accumulation-group completions
    sc_sem = sem("sc_sem")  # scalar silu completions
    vec_sem = sem("vec_sem")  # vector psum->sbuf copy completions
    # Per-ob-slot output-DMA sems: a slot has at most one DMA in flight
    # (the next one is gated on the slot's own copy chain), so cumulative
    # per-slot thresholds are exact.
    od_sems = [sem(f"od_sem{i}") for i in range(4)]
    ms_sem = sem("ms_sem")  # warm-tile memset

    all_sems = (
        tk_sems
        + w1_sems
        + w2_sems
        + [pe_sem, sc_sem, vec_sem, ms_sem]
        + od_sems
    )

    silu = mybir.ActivationFunctionType.Silu

    # =================== sync engine (SP HWDGE ring) ===================
    # tok0 first (startup-critical), then the odd W1 slabs interleaved in
    # fj order, then tok1 (not needed until stage1 of chunk 1, ~120us in).
    nc.sync.dma_start(tok_sb[0][:], toks[0][:]).then_inc(tk_sems[0], 16)
    for i in range(1, W1_NSLAB, 2):
        nc.sync.dma_start(w1_sb[i][:], w1s[i][:]).then_inc(w1_sems[i], 16)
    nc.sync.dma_start(tok_sb[1][:], toks[1][:]).then_inc(tk_sems[1], 16)
    # W2 slabs ride both rings BEHIND everything startup-critical: ring
    # FIFO defers them until W1/tokens are done, and they still land
    # ~20us+ before stage 2 needs them.
    for i in range(1, len(w2s), 2):
        nc.sync.dma_start(w2_sb[i][:], w2s[i][:]).then_inc(w2_sems[i], 16)

    # output DMAs (emitted now; they execute in order, each gated on its
    # copy).  Last group's output is split in two so the final transfer
    # (the kernel tail) is half-size and overlaps the second copy.
    for c in range(NCHUNK):
        for dm in range(KD):
            g2 = c * KD + dm
            if g2 < NCHUNK * KD - 1:
                nc.sync.wait_ge(vec_sem, g2 + 1)
                nc.sync.dma_start(
                    out[dm * P : (dm + 1) * P, c * CH : (c + 1) * CH],
                    ob_sb[:, g2 % 4, :],
                ).then_inc(od_sems[g2 % 4], 16)
            else:
                half = CH // 2
                for hh in range(2):
                    nc.sync.wait_ge(vec_sem, g2 + 1 + hh)
                    nc.sync.dma_start(
                        out[
                            dm * P : (dm + 1) * P,
                            c * CH + hh * half : c * CH + (hh + 1) * half,
                        ],
                        ob_sb[:, g2 % 4, hh * half : (hh + 1) * half],
                    ).then_inc(od_sems[g2 % 4], 16)
    for s in range(4):
        n_dmas = len([g for g in range(NCHUNK * KD) if g % 4 == s])
        if s == (NCHUNK * KD - 1) % 4:
            n_dmas += 1  # split last group
        nc.sync.wait_ge(od_sems[s], 16 * n_dmas)

    # =================== scalar engine (ACT HWDGE ring) ===================
    for i in range(0, W1_NSLAB, 2):
        nc.scalar.dma_start(w1_sb[i][:], w1s[i][:]).then_inc(w1_sems[i], 16)
    for i in range(0, len(w2s), 2):
        nc.scalar.dma_start(w2_sb[i][:], w2s[i][:]).then_inc(w2_sems[i], 16)

    for c in range(NCHUNK):
        for fj in range(KF):
            s_glob = c * KF + fj
            nc.scalar.wait_ge(pe_sem, c * 24 + fj // 2 + 1)
            nc.scalar.activation(
                act_sb[:, fj, :], ps1[s_glob % 4][:], silu
            ).then_inc(sc_sem, 1)

    # =================== gpsimd: warm tile + W2 via SWDGE ===================
    # W2 rides the third (SWDGE) ring, gated behind the token loads so it
    # does not steal SDMA bandwidth from the startup-critical tok0/W1 path.
    # It is needed ~55us after the tokens land — lots of slack.
    # Clear this kernel's semaphores first: the runtime does not guarantee
    # initial sem state, and every real wait happens >=6us after these.
    from concourse.bass import compact_to_ranges as _ctr

    for rng in _ctr([s.num for s in all_sems]):
        nc.gpsimd.dma_reset(rng)
        nc.gpsimd.sem_clear(rng)
    nc.gpsimd.memset(warm_sb[:], 0.0).then_inc(ms_sem, 1)

    # =================== tensor engine ===================
    # warm-up: keep PE busy during the input-DMA wait so the HAM clock
    # gate reaches 8/8 before real matmuls start.
    nc.tensor.wait_ge(ms_sem, 1)
    for _ in range(N_WARM):
        nc.tensor.matmul(warm_ps[:, 0:P], warm_sb[:], warm_sb[:], start=True, stop=True)

    for c in range(NCHUNK):
        # ---- stage 1: act[f, :] = silu(sum_d w1[d, f] * tok[d, :]) ----
        for fj in range(KF):
            s_glob = c * KF + fj
            sl = _w1_slab_of(fj)
            col = fj * P - sl * W1_COLS
            if fj == 0:
                nc.tensor.wait_ge(tk_sems[c], 16)
            if fj == 0 or _w1_slab_of(fj - 1) != sl:
                nc.tensor.wait_ge(w1_sems[sl], 16)
            if s_glob >= 4:  # ps1 bank WAR vs silu of group s_glob-4
                nc.tensor.wait_ge(sc_sem, s_glob - 3)
            for dk in range(KD):
                mm = nc.tensor.matmul(
                    ps1[s_glob % 4][:],
                    w1_sb[sl][:, dk, col : col + P],
                    tok_sb[c][:, dk, :],
                    start=(dk == 0),
                    stop=(dk == KD - 1),
                )
            if fj % 2 == 1:  # one inc per 2 groups (a PE sem-write costs ~31ns)
                mm.then_inc(pe_sem, 1)
        # ---- stage 2: out[d, :] = sum_f w2[f, d] * act[f, :] ----
        for dm in range(KD):
            g2 = c * KD + dm
            if dm % 2 == 0:
                nc.tensor.wait_ge(w2_sems[dm // 2], 16)
            if g2 >= 2:  # ps2 bank WAR vs copy of group g2-2
                nc.tensor.wait_ge(vec_sem, g2 - 1)
            for fk in range(KF):
                # act readiness: a wait every 4th MM (covering the next 4
                # silus) tracks the producer closely without stalling and
                # without one multi-wait event-sem per matmul.
                if dm == 0 and fk % 4 == 0:
                    nc.tensor.wait_ge(sc_sem, c * KF + min(fk + 4, KF))
                mm = nc.tensor.matmul(
                    ps2[g2 % 2][:],
                    w2_sb[dm // 2][:, fk, (dm % 2) * P : (dm % 2) * P + P],
                    act_sb[:, fk, :],
                    start=(fk == 0),
                    stop=(fk == KF - 1),
                )
            mm.then_inc(pe_sem, 1)

    # =================== vector engine ===================
    for c in range(NCHUNK):
        for dm in range(KD):
            g2 = c * KD + dm
            nc.vector.wait_ge(pe_sem, c * 24 + 16 + dm + 1)
            if g2 >= 4:  # ob buffer WAR vs out-DMA of copy g2-4
                nc.vector.wait_ge(od_sems[g2 % 4], 16 * ((g2 - 4) // 4 + 1))
            if g2 < NCHUNK * KD - 1:
                nc.vector.tensor_copy(ob_sb[:, g2 % 4, :], ps2[g2 % 2][:]).then_inc(
                    vec_sem, 1
                )
            else:  # split last copy so the tail DMA starts sooner
                half = CH // 2
                for hh in range(2):
                    nc.vector.tensor_copy(
                        ob_sb[:, g2 % 4, hh * half : (hh + 1) * half],
                        ps2[g2 % 2][:, hh * half : (hh + 1) * half],
                    ).then_inc(vec_sem, 1)

    nc.compile()
    stack.close()
    return nc


def _get_nc():
    if "nc" not in _nc_cache:
        _nc_cache["nc"] = _build()
    return _nc_cache["nc"]


def kernel(**inputs) -> np.ndarray:
    global LAST_RESULTS
    x = np.asarray(inputs["x"], dtype=np.float32)
    Wg = np.asarray(inputs["Wg"], dtype=np.float32)
    W1 = np.asarray(inputs["W1"], dtype=np.float32)
    W2 = np.asarray(inputs["W2"], dtype=np.float32)

    h = np.ascontiguousarray(x.reshape(N, D))

    # ---- host gate: top-2 + softmax (0.05% of total FLOPs) ----
    logits = h @ Wg.T  # [N, E] f32
    idx2 = np.argpartition(-logits, 1, axis=1)[:, :2]
    lsel = np.take_along_axis(logits, idx2, axis=1)
    first = lsel[:, 0] >= lsel[:, 1]
    i0 = np.where(first, idx2[:, 0], idx2[:, 1])
    i1 = np.where(first, idx2[:, 1], idx2[:, 0])
    l0 = np.where(first, lsel[:, 0], lsel[:, 1])
    l1 = np.where(first, lsel[:, 1], lsel[:, 0])
    e1 = np.exp((l1 - l0).astype(np.float32))
    w0 = (1.0 / (1.0 + e1)).astype(np.float32)
    w1g = (e1 / (1.0 + e1)).astype(np.float32)

    token_ids = np.concatenate([np.arange(N), np.arange(N)])
    expert_ids = np.concatenate([i0, i1])
    gate_w = np.concatenate([w0, w1g])

    hb = h.astype(bf16)
    W1b = W1.astype(bf16)
    W2b = W2.astype(bf16)

    y = np.zeros((N, D), dtype=np.float32)

    in_maps = []
    ids_per_expert = []
    gw_per_expert = []
    for e in range(E):
        sel = np.flatnonzero(expert_ids == e)
        ids_e = token_ids[sel]
        gw_e = gate_w[sel]
        if len(ids_e) > C:
            # capacity overflow -> host fp32 FFN, folded into scatter-add
            ov_ids = ids_e[C:]
            ov_gw = gw_e[C:]
            up = h[ov_ids] @ W1[e]
            act = up * (1.0 / (1.0 + np.exp(-up)))
            y[ov_ids] += ov_gw[:, None] * (act @ W2[e])
            ids_e = ids_e[:C]
            gw_e = gw_e[:C]
        n_e = len(ids_e)
        ids_per_expert.append(ids_e)
        gw_per_expert.append(gw_e)

        tokT = np.zeros((P, KD, C), dtype=bf16)
        # tokens [n,D] -> [D,n] -> [KD,P,n] -> [P,KD,n]
        tokT[:, :, :n_e] = hb[ids_e].T.reshape(KD, P, n_e).transpose(1, 0, 2)
        W1pe = W1b[e].reshape(KD, P, F).transpose(1, 0, 2)  # [P, KD, F]
        W2pe = W2b[e].reshape(KF, P, D).transpose(1, 0, 2)  # [P, KF, D]
        m = {}
        for c in range(NCHUNK):
            m[f"tok{c}"] = np.ascontiguousarray(tokT[:, :, c * CH : (c + 1) * CH])
        for i in range(W1_NSLAB):
            m[f"w1s{i}"] = np.ascontiguousarray(
                W1pe[:, :, i * W1_COLS : (i + 1) * W1_COLS]
            )
        for i in range(D // W2_SLAB):
            m[f"w2s{i}"] = np.ascontiguousarray(
                W2pe[:, :, i * W2_SLAB : (i + 1) * W2_SLAB]
            )
        in_maps.append(m)

    nc = _get_nc()
    from concourse.bass_utils import run_bass_kernel_spmd

    LAST_RESULTS = run_bass_kernel_spmd(
        nc, in_maps, core_ids=list(range(E)), trace=TRACE
    )

    for e in range(E):
        o = np.asarray(LAST_RESULTS.results[e]["out"], dtype=np.float32)  # [D, C]
        ids_e = ids_per_expert[e]
        n_e = len(ids_e)
        y[ids_e] += gw_per_expert[e][:, None] * o[:, :n_e].T
    return y.reshape(B, T, D)


# revision 45
# speedup vs baseline: 1.0021x; 1.0021x over previous
"""Trainium2 Bass kernel for nn_MoELayer (top-2 MoE, E=8 experts).

Strategy (expert-parallel across 8 NeuronCores):
  - Host computes the (tiny) gate matmul + top-2 + softmax and dispatches
    each token to its two experts' cores. One expert per core.
  - Token capacity per core is fixed at C=1024 (2 PSUM-friendly chunks of
    512). The few overflow tokens of experts routed >1024 tokens (~1% of
    FLOPs) are computed on the host in fp32 and folded into the
    scatter-add, keeping every core's device work identical and minimal.
  - Each core runs a RAW-BASS (hand-synchronized) kernel computing, for
    its expert e and its <=1024 routed tokens:
        out = silu(tok @ W1[e]) @ W2[e]
    with bf16 matmul inputs and fp32 PSUM accumulation.

Why raw bass instead of Tile: Tile attaches a semaphore increment to
EVERY matmul (each accumulation-group member has the PSUM-reading
activation as a descendant).  A PE-engine semaphore update costs ~31ns
on the engine timeline (SEM_PROP_BASE 17 + PE send overhead 14), which
measured as a ~33ns/MM tax (~48us total) in the Tile baseline.  Here
semaphores are placed by hand: one inc per 8-matmul accumulation group.

Layouts (no on-device transposes):
  stage 1:  actT[f, c] = silu( sum_d W1[d, f] * tokT[d, c] )
  stage 2:  outT[d, c] = sum_f W2[f, d] * actT[f, c]

DMA plan: W1 streams on the scalar-engine HWDGE ring in 4 fj-ordered
slabs (so the first matmul only waits for tok chunk 0 + slab 0), while
tokens/W2 stream on the sync-engine ring.  Every input DMA is a single
contiguous-per-partition block (own DRAM tensor per slab) for >=4KB
descriptors (full DMA line rate).  ~55 throwaway warm-up matmuls run
during the initial DMA wait to lift the PE HAM clock-gate to 2.4GHz
before real work starts.
"""

import math
import os
import sys

sys.path.insert(0, "/opt/trn_rl_repo")

import ml_dtypes
import numpy as np

B, T, D, F, E = 2, 2048, 1024, 4096, 8
N = B * T
P = 128
KD = D // P  # 8
KF = F // P  # 32

C = 1024  # device token capacity per expert/core
CH = 512  # chunk width (PSUM bank = 512 fp32)
NCHUNK = C // CH  # 2

# W1 streamed in 16 fj-ordered 256-col slabs (0.5MB each), alternating
# between the two HWDGE rings (scalar gets even slabs, sync odd) — one
# ring sustains only ~170GB/s, which cannot keep ahead of the PE's
# 154GB/s W1 burn once the startup path is fast; two rings give 2x margin
# and the first matmul only waits for tok0 + one 0.5MB slab.
W1_NSLAB = 16
W1_COLS = F // W1_NSLAB  # 256
W2_SLAB = 256  # D-cols per W2 slab (4 slabs)
N_WARM = int(os.environ.get("N_WARM", "100"))

bf16 = ml_dtypes.bfloat16

_nc_cache: dict = {}
LAST_RESULTS = None  # BassKernelResults from the most recent run (for test.py)
TRACE = False


def _w1_slab_of(fj: int) -> int:
    return (fj * P) // W1_COLS


def _build():
    import concourse.mybir as mybir
    from concourse import bacc

    dt = mybir.dt
    nc = bacc.Bacc(None, target_bir_lowering=False)

    # ---- DRAM tensors (each a contiguous block => big DMA descriptors) ----
    # tok0 ships as two dk-halves (contiguous 4KB/partition runs) so its
    # startup-critical load rides BOTH HWDGE rings; tok1 is one block.
    tok0h = [
        nc.dram_tensor(
            f"tok0h{i}", [P, KD // 2, CH], dt.bfloat16, kind="ExternalInput"
        )
        for i in range(2)
    ]
    tok1 = nc.dram_tensor("tok1", [P, KD, CH], dt.bfloat16, kind="ExternalInput")
    w1s = [
        nc.dram_tensor(f"w1s{i}", [P, KD, W1_COLS], dt.bfloat16, kind="ExternalInput")
        for i in range(W1_NSLAB)
    ]
    w2s = [
        nc.dram_tensor(f"w2s{i}", [P, KF, W2_SLAB], dt.bfloat16, kind="ExternalInput")
        for i in range(D // W2_SLAB)
    ]
    out = nc.dram_tensor("out", [D, C], dt.float32, kind="ExternalOutput")

    from contextlib import ExitStack

    stack = ExitStack()
    sb = lambda name, shape, dty: stack.enter_context(nc.sbuf_tensor(name, shape, dty))
    ps = lambda name, shape: stack.enter_context(
        nc.psum_tensor(name, shape, dt.float32)
    )
    sem = lambda name: stack.enter_context(nc.semaphore(name))

    tok_sb = [sb(f"tok_sb{c}", [P, KD, CH], dt.bfloat16) for c in range(NCHUNK)]
    w1_sb = [
        sb(f"w1_sb{i}", [P, KD, W1_COLS], dt.bfloat16) for i in range(W1_NSLAB)
    ]
    w2_sb = [sb(f"w2_sb{i}", [P, KF, W2_SLAB], dt.bfloat16) for i in range(len(w2s))]
    act_sb = sb("act_sb", [P, KF, CH], dt.bfloat16)
    ob_sb = sb("ob_sb", [P, 4, CH], dt.float32)  # 4 rotating output buffers
    warm_sb = sb("warm_sb", [P, P], dt.bfloat16)

    ps1 = [ps(f"ps1_{b}", [P, CH]) for b in range(4)]  # stage-1 banks
    ps2 = [ps(f"ps2_{b}", [P, CH]) for b in range(2)]  # stage-2 banks
    warm_ps = ps("warm_ps", [P, CH])

    # One semaphore per input DMA: a DMA's completion is 16 per-engine
    # increments, and increments of DIFFERENT in-flight DMAs sharing one
    # semaphore interleave under engine skew — a cumulative threshold can
    # fire while an earlier DMA is still partially in flight.  Dedicated
    # sems (wait >= 16) are skew-proof.
    tk_sems = [sem(f"tk_sem{c}") for c in range(3)]  # tok0a, tok0b, tok1
    w1_sems = [sem(f"w1_sem{i}") for i in range(W1_NSLAB)]
    w2_sems = [sem(f"w2_sem{i}") for i in range(len(w2s))]
    pe_sem = sem("pe_sem")  # PE accumulation-group completions
    sc_sem = sem("sc_sem")  # scalar silu completions
    vec_sem = sem("vec_sem")  # vector psum->sbuf copy completions
    # Per-ob-slot output-DMA sems: a slot has at most one DMA in flight
    # (the next one is gated on the slot's own copy chain), so cumulative
    # per-slot thresholds are exact.
    od_sems = [sem(f"od_sem{i}") for i in range(4)]
    ms_sem = sem("ms_sem")  # warm-tile memset

    all_sems = (
        tk_sems
        + w1_sems
        + w2_sems
        + [pe_sem, sc_sem, vec_sem, ms_sem]
        + od_sems
    )

    silu = mybir.ActivationFunctionType.Silu

    # =================== sync engine (SP HWDGE ring) ===================
    # tok0 first (startup-critical), then the odd W1 slabs interleaved in
    # fj order, then tok1 (not needed until stage1 of chunk 1, ~120us in).
    nc.sync.dma_start(tok_sb[0][:, 0 : KD // 2, :], tok0h[0][:]).then_inc(
        tk_sems[0], 16
    )
    nc.sync.dma_start(w1_sb[0][:], w1s[0][:]).then_inc(w1_sems[0], 16)
    for i in range(1, W1_NSLAB, 2):
        nc.sync.dma_start(w1_sb[i][:], w1s[i][:]).then_inc(w1_sems[i], 16)
    nc.sync.dma_start(tok_sb[1][:], tok1[:]).then_inc(tk_sems[2], 16)
    # W2 slabs ride both rings BEHIND everything startup-critical: ring
    # FIFO defers them until W1/tokens are done, and they still land
    # ~20us+ before stage 2 needs them.
    for i in range(1, len(w2s), 2):
        nc.sync.dma_start(w2_sb[i][:], w2s[i][:]).then_inc(w2_sems[i], 16)

    # output DMAs (emitted now; they execute in order, each gated on its
    # copy).  Last group's output is split in two so the final transfer
    # (the kernel tail) is half-size and overlaps the second copy.
    for c in range(NCHUNK):
        for dm in range(KD):
            g2 = c * KD + dm
            if g2 < NCHUNK * KD - 1:
                nc.sync.wait_ge(vec_sem, g2 + 1)
                nc.sync.dma_start(
                    out[dm * P : (dm + 1) * P, c * CH : (c + 1) * CH],
                    ob_sb[:, g2 % 4, :],
                ).then_inc(od_sems[g2 % 4], 16)
            else:
                half = CH // 2
                for hh in range(2):
                    nc.sync.wait_ge(vec_sem, g2 + 1 + hh)
                    nc.sync.dma_start(
                        out[
                            dm * P : (dm + 1) * P,
                            c * CH + hh * half : c * CH + (hh + 1) * half,
                        ],
                        ob_sb[:, g2 % 4, hh * half : (hh + 1) * half],
                    ).then_inc(od_sems[g2 % 4], 16)
    for s in range(4):
        n_dmas = len([g for g in range(NCHUNK * KD) if g % 4 == s])
        if s == (NCHUNK * KD - 1) % 4:
            n_dmas += 1  # split last group
        nc.sync.wait_ge(od_sems[s], 16 * n_dmas)

    # =================== scalar engine (ACT HWDGE ring) ===================
    nc.scalar.dma_start(tok_sb[0][:, KD // 2 :, :], tok0h[1][:]).then_inc(
        tk_sems[1], 16
    )
    for i in range(2, W1_NSLAB, 2):
        nc.scalar.dma_start(w1_sb[i][:], w1s[i][:]).then_inc(w1_sems[i], 16)
    for i in range(0, len(w2s), 2):
        nc.scalar.dma_start(w2_sb[i][:], w2s[i][:]).then_inc(w2_sems[i], 16)

    for c in range(NCHUNK):
        for fj in range(KF):
            s_glob = c * KF + fj
            nc.scalar.wait_ge(pe_sem, c * 24 + fj // 2 + 1)
            nc.scalar.activation(
                act_sb[:, fj, :], ps1[s_glob % 4][:], silu
            ).then_inc(sc_sem, 1)

    # =================== gpsimd: warm tile + W2 via SWDGE ===================
    # W2 rides the third (SWDGE) ring, gated behind the token loads so it
    # does not steal SDMA bandwidth from the startup-critical tok0/W1 path.
    # It is needed ~55us after the tokens land — lots of slack.
    # Clear this kernel's semaphores first: the runtime does not guarantee
    # initial sem state, and every real wait happens >=6us after these.
    from concourse.bass import compact_to_ranges as _ctr

    for rng in _ctr([s.num for s in all_sems]):
        nc.gpsimd.dma_reset(rng)
        nc.gpsimd.sem_clear(rng)
    nc.gpsimd.memset(warm_sb[:], 0.0).then_inc(ms_sem, 1)

    # =================== tensor engine ===================
    # warm-up: keep PE busy during the input-DMA wait so the HAM clock
    # gate reaches 8/8 before real matmuls start.
    nc.tensor.wait_ge(ms_sem, 1)
    for _ in range(N_WARM):
        nc.tensor.matmul(warm_ps[:, 0:P], warm_sb[:], warm_sb[:], start=True, stop=True)

    for c in range(NCHUNK):
        # ---- stage 1: act[f, :] = silu(sum_d w1[d, f] * tok[d, :]) ----
        for fj in range(KF):
            s_glob = c * KF + fj
            sl = _w1_slab_of(fj)
            col = fj * P - sl * W1_COLS
            if fj == 0:
                if c == 0:
                    nc.tensor.wait_ge(tk_sems[0], 16)
                    nc.tensor.wait_ge(tk_sems[1], 16)
                else:
                    nc.tensor.wait_ge(tk_sems[2], 16)
            if fj == 0 or _w1_slab_of(fj - 1) != sl:
                nc.tensor.wait_ge(w1_sems[sl], 16)
            if s_glob >= 4:  # ps1 bank WAR vs silu of group s_glob-4
                nc.tensor.wait_ge(sc_sem, s_glob - 3)
            for dk in range(KD):
                mm = nc.tensor.matmul(
                    ps1[s_glob % 4][:],
                    w1_sb[sl][:, dk, col : col + P],
                    tok_sb[c][:, dk, :],
                    start=(dk == 0),
                    stop=(dk == KD - 1),
                )
            if fj % 2 == 1:  # one inc per 2 groups (a PE sem-write costs ~31ns)
                mm.then_inc(pe_sem, 1)
        # ---- stage 2: out[d, :] = sum_f w2[f, d] * act[f, :] ----
        for dm in range(KD):
            g2 = c * KD + dm
            if dm % 2 == 0:
                nc.tensor.wait_ge(w2_sems[dm // 2], 16)
            if g2 == NCHUNK * KD - 1:
                # Last group runs as two N=256 half-groups on alternating
                # banks so the first half's copy+DMA overlaps the second
                # half's matmuls, shrinking the kernel tail.
                half = CH // 2
                for hh in range(2):
                    nc.tensor.wait_ge(vec_sem, g2 - 1 + hh)
                    for fk in range(KF):
                        mm = nc.tensor.matmul(
                            ps2[(g2 + hh) % 2][:, 0:half],
                            w2_sb[dm // 2][:, fk, (dm % 2) * P : (dm % 2) * P + P],
                            act_sb[:, fk, hh * half : (hh + 1) * half],
                            start=(fk == 0),
                            stop=(fk == KF - 1),
                        )
                    mm.then_inc(pe_sem, 1)
                continue
            if g2 >= 2:  # ps2 bank WAR vs copy of group g2-2
                nc.tensor.wait_ge(vec_sem, g2 - 1)
            for fk in range(KF):
                # act readiness: a wait every 4th MM (covering the next 4
                # silus) tracks the producer closely without stalling and
                # without one multi-wait event-sem per matmul.
                if dm == 0 and fk % 4 == 0:
                    nc.tensor.wait_ge(sc_sem, c * KF + min(fk + 4, KF))
                mm = nc.tensor.matmul(
                    ps2[g2 % 2][:],
                    w2_sb[dm // 2][:, fk, (dm % 2) * P : (dm % 2) * P + P],
                    act_sb[:, fk, :],
                    start=(fk == 0),
                    stop=(fk == KF - 1),
                )
            mm.then_inc(pe_sem, 1)

    # =================== vector engine ===================
    for c in range(NCHUNK):
        for dm in range(KD):
            g2 = c * KD + dm
            if g2 >= 4:  # ob buffer WAR vs out-DMA of copy g2-4
                nc.vector.wait_ge(od_sems[g2 % 4], 16 * ((g2 - 4) // 4 + 1))
            if g2 < NCHUNK * KD - 1:
                nc.vector.wait_ge(pe_sem, c * 24 + 16 + dm + 1)
                nc.vector.tensor_copy(ob_sb[:, g2 % 4, :], ps2[g2 % 2][:]).then_inc(
                    vec_sem, 1
                )
            else:  # two half-copies chasing the two half-groups
                half = CH // 2
                for hh in range(2):
                    nc.vector.wait_ge(pe_sem, c * 24 + 16 + dm + 1 + hh)
                    nc.vector.tensor_copy(
                        ob_sb[:, g2 % 4, hh * half : (hh + 1) * half],
                        ps2[(g2 + hh) % 2][:, 0:half],
                    ).then_inc(vec_sem, 1)

    nc.compile()
    stack.close()
    return nc


def _get_nc():
    if "nc" not in _nc_cache:
        _nc_cache["nc"] = _build()
    return _nc_cache["nc"]


def kernel(**inputs) -> np.ndarray:
    global LAST_RESULTS
    x = np.asarray(inputs["x"], dtype=np.float32)
    Wg = np.asarray(inputs["Wg"], dtype=np.float32)
    W1 = np.asarray(inputs["W1"], dtype=np.float32)
    W2 = np.asarray(inputs["W2"], dtype=np.float32)

    h = np.ascontiguousarray(x.reshape(N, D))

    # ---- host gate: top-2 + softmax (0.05% of total FLOPs) ----
    logits = h @ Wg.T  # [N, E] f32
    idx2 = np.argpartition(-logits, 1, axis=1)[:, :2]
    lsel = np.take_along_axis(logits, idx2, axis=1)
    first = lsel[:, 0] >= lsel[:, 1]
    i0 = np.where(first, idx2[:, 0], idx2[:, 1])
    i1 = np.where(first, idx2[:, 1], idx2[:, 0])
    l0 = np.where(first, lsel[:, 0], lsel[:, 1])
    l1 = np.where(first, lsel[:, 1], lsel[:, 0])
    e1 = np.exp((l1 - l0).astype(np.float32))
    w0 = (1.0 / (1.0 + e1)).astype(np.float32)
    w1g = (e1 / (1.0 + e1)).astype(np.float32)

    token_ids = np.concatenate([np.arange(N), np.arange(N)])
    expert_ids = np.concatenate([i0, i1])
    gate_w = np.concatenate([w0, w1g])

    hb = h.astype(bf16)
    W1b = W1.astype(bf16)
    W2b = W2.astype(bf16)

    y = np.zeros((N, D), dtype=np.float32)

    in_maps = []
    ids_per_expert = []
    gw_per_expert = []
    for e in range(E):
        sel = np.flatnonzero(expert_ids == e)
        ids_e = token_ids[sel]
        gw_e = gate_w[sel]
        if len(ids_e) > C:
            # capacity overflow -> host fp32 FFN, folded into scatter-add
            ov_ids = ids_e[C:]
            ov_gw = gw_e[C:]
            up = h[ov_ids] @ W1[e]
            act = up * (1.0 / (1.0 + np.exp(-up)))
            y[ov_ids] += ov_gw[:, None] * (act @ W2[e])
            ids_e = ids_e[:C]
            gw_e = gw_e[:C]
        n_e = len(ids_e)
        ids_per_expert.append(ids_e)
        gw_per_expert.append(gw_e)

        tokT = np.zeros((P, KD, C), dtype=bf16)
        # tokens [n,D] -> [D,n] -> [KD,P,n] -> [P,KD,n]
        tokT[:, :, :n_e] = hb[ids_e].T.reshape(KD, P, n_e).transpose(1, 0, 2)
        W1pe = W1b[e].reshape(KD, P, F).transpose(1, 0, 2)  # [P, KD, F]
        W2pe = W2b[e].reshape(KF, P, D).transpose(1, 0, 2)  # [P, KF, D]
        m = {}
        m["tok0h0"] = np.ascontiguousarray(tokT[:, 0 : KD // 2, 0:CH])
        m["tok0h1"] = np.ascontiguousarray(tokT[:, KD // 2 :, 0:CH])
        m["tok1"] = np.ascontiguousarray(tokT[:, :, CH : 2 * CH])
        for i in range(W1_NSLAB):
            m[f"w1s{i}"] = np.ascontiguousarray(
                W1pe[:, :, i * W1_COLS : (i + 1) * W1_COLS]
            )
        for i in range(D // W2_SLAB):
            m[f"w2s{i}"] = np.ascontiguousarray(
                W2pe[:, :, i * W2_SLAB : (i + 1) * W2_SLAB]
            )
        in_maps.append(m)

    nc = _get_nc()
    from concourse.bass_utils import run_bass_kernel_spmd

    LAST_RESULTS = run_bass_kernel_spmd(
        nc, in_maps, core_ids=list(range(E)), trace=TRACE
    )

    for e in range(E):
        o = np.asarray(LAST_RESULTS.results[e]["out"], dtype=np.float32)  # [D, C]
        ids_e = ids_per_expert[e]
        n_e = len(ids_e)
        y[ids_e] += gw_per_expert[e][:, None] * o[:, :n_e].T
    return y.reshape(B, T, D)
